# revision 39
# baseline (speedup 1.0000x reference)
"""AttentionXL Trainium2 kernel: 8-core = 4 batch x 2 head-group parallel.

Self-contained: hardcodes shapes from the problem spec.
  inputs:       (1024, 4, 1024) f32   cur_seq, bs, d
  full_input:   (2048, 4, 1024) f32   full_seq, bs, d
  pos_embedding:(2048, 1024)    f32
  u, v:         (16, 64)        f32   H, D
  Wkv (1024, 2*1024), Wq/Wr/Wo (1024, 1024), biases zero, mask all-False.

Per-core (batch b = core//2, head group g = core%2 -> heads 8g..8g+7),
all bf16 matmuls. Per head h (8 pairs, software-pipelined: the next
pair's BD matmuls/writes are interleaved into the current pair's S/AV
loop to keep the PE queue full):
  BD  = (q+v)^T r  in [i, j] layout -> DRAM scratch with row pitch fs+1
  BDshifted^T <- DMA-transpose read of the pitched buffer (reproduces the
     reference rel_shift flat-reinterpret exactly, incl. wrap)
  S^T = K^T(q+u) + I @ BDshifted^T   (PSUM accumulate)
  E^T = exp(S^T / 8)  (ScalarE eviction, bf16)
  O^T[65] = [V | 1]^T E^T  (AV accumulated over j tiles; row 64 = denom)
  attn_vec = O^T[0:64] * (1/Z) ; y_partial = Wo_g^T attn_vec -> DRAM
DMA queue split (big win on HW): p2 writes + x loads + y writes issue from
the idle Pool/gpsimd queue; the SP queue carries only the transposed
shifted reads, so they are never stuck behind bulk writes.
Host: sum the two head-group partials per batch, add bo.
Env knobs (default = fast path; used for ablations): KABL, KILV, KPSA,
KPSX, KQIO.
"""

import os
from contextlib import ExitStack

import numpy as np
import ml_dtypes

import concourse.bass as bass
import concourse.bacc as bacc_mod
import concourse.mybir as mybir
import concourse.tile as tile
from concourse.masks import make_identity

BF16 = mybir.dt.bfloat16
F32 = mybir.dt.float32
FP8 = mybir.dt.float8e4
NPBF16 = ml_dtypes.bfloat16
NPFP8 = ml_dtypes.float8_e4m3

# Problem dims (full size)
CS, FS, BS, D_MODEL = 1024, 2048, 4, 1024
H, HD = 16, 64
N_CORES = 8
NB, NG = 4, 2               # batch split x head-group split
HPC = H // NG               # heads per core = 8
DC = HPC * HD               # per-core model slice = 512
WS = 64.0                   # fp8 weight pre-scale


def build_core_kernel(cs=CS, fs=FS, bs=BS, d=D_MODEL, hpc=HPC, hd=HD, loop=1):
    dc = hpc * hd           # 512
    not_ = dc // 128        # output tiles per projection = 4
    nk2 = d // 256          # fp8 DoubleRow contraction chunks = 4
    nk = d // 128           # bf16 contraction chunks = 8
    NI = cs // 128          # 8
    NJ = fs // 128          # 16
    NJC = fs // 512         # 4
    NIC = cs // 512         # 2
    scale = 1.0 / (hd ** 0.5)

    nc = bacc_mod.Bacc(None, target_bir_lowering=False, debug=False)

    xcur = nc.dram_tensor("xcur", [d, cs], BF16, kind="ExternalInput")
    xfull = nc.dram_tensor("xfull", [d, fs], BF16, kind="ExternalInput")
    posT = nc.dram_tensor("posT", [d, fs], BF16, kind="ExternalInput")
    wq_d = nc.dram_tensor("wq", [d, dc], BF16, kind="ExternalInput")
    wk_d = nc.dram_tensor("wk", [d, dc], BF16, kind="ExternalInput")
    wr_d = nc.dram_tensor("wr", [d, dc], BF16, kind="ExternalInput")
    wv_d = nc.dram_tensor("wv", [d, dc], BF16, kind="ExternalInput")
    wo_d = nc.dram_tensor("wo", [dc, d], BF16, kind="ExternalInput")
    u_d = nc.dram_tensor("u", [dc, 1], F32, kind="ExternalInput")
    v_d = nc.dram_tensor("v", [dc, 1], F32, kind="ExternalInput")
    y_d = nc.dram_tensor("y", [d, cs], BF16, kind="ExternalOutput")

    # DRAM scratch for the rel-shift pitch trick, one per in-flight pair.
    p2 = [nc.dram_tensor(f"p2_{i}", [cs * (fs + 1)], BF16) for i in range(4)]

    with tile.TileContext(nc) as tc, ExitStack() as ctx:
        const = ctx.enter_context(tc.tile_pool(name="const", bufs=1))
        persist = ctx.enter_context(tc.tile_pool(name="persist", bufs=1))
        xqp = ctx.enter_context(tc.tile_pool(name="xqp", bufs=10))
        xrp = ctx.enter_context(tc.tile_pool(name="xrp", bufs=9))
        xkv = ctx.enter_context(tc.tile_pool(name="xkv", bufs=9))
        sts = ctx.enter_context(tc.tile_pool(name="sts", bufs=3))
        bdst = ctx.enter_context(tc.tile_pool(name="bdst", bufs=10))
        ea = ctx.enter_context(tc.tile_pool(name="ea", bufs=2))
        vxp = ctx.enter_context(tc.tile_pool(name="vxp", bufs=3))
        onrm = ctx.enter_context(tc.tile_pool(name="onrm", bufs=1))
        yout = ctx.enter_context(tc.tile_pool(name="yout", bufs=2))
        _psa = int(os.environ.get("KPSA", "3"))
        _psx = int(os.environ.get("KPSX", "1"))
        psA = ctx.enter_context(
            tc.tile_pool(name="psA", bufs=_psa, space="PSUM"))
        psB = ctx.enter_context(tc.tile_pool(name="psB", bufs=2, space="PSUM"))
        psO = ctx.enter_context(tc.tile_pool(name="psO", bufs=1, space="PSUM"))
        psX = ctx.enter_context(
            tc.tile_pool(name="psX", bufs=_psx, space="PSUM"))

        # ---- constants / weights in SBUF ----
        ident = const.tile([128, 128], BF16)
        make_identity(nc, ident[:])

        def load_w(dram, nm):
            # [d, dc] -> SBUF [128, nk, dc]; contraction row = 128*kk + p.
            t = const.tile([128, nk * dc], BF16, name=nm, tag=nm)
            src = bass.AP(tensor=dram, offset=0,
                          ap=[[dc, 128], [128 * dc, nk], [1, dc]])
            nc.sync.dma_start(out=t[:], in_=src)
            return t[:].rearrange("p (kk c) -> p kk c", kk=nk)

        wq = load_w(wq_d, "wq_sb")
        wk = load_w(wk_d, "wk_sb")
        wr = load_w(wr_d, "wr_sb")
        wv = load_w(wv_d, "wv_sb")
        wo_t = const.tile([128, not_ * d], BF16)
        nc.sync.dma_start(
            out=wo_t[:],
            in_=bass.AP(tensor=wo_d, offset=0,
                        ap=[[d, 128], [128 * d, not_], [1, d]]))
        wo = wo_t[:].rearrange("p (kt c) -> p kt c", kt=not_)
        u_sb = const.tile([128, not_], F32)
        v_sb = const.tile([128, not_], F32)
        nc.sync.dma_start(
            out=u_sb[:], in_=bass.AP(tensor=u_d, offset=0,
                                     ap=[[1, 128], [128, not_]]))
        nc.sync.dma_start(
            out=v_sb[:], in_=bass.AP(tensor=v_d, offset=0,
                                     ap=[[1, 128], [128, not_]]))

        # zero column 0 of each p2 buffer
        zc = cs // 128
        zcol = const.tile([128, zc], BF16)
        nc.vector.memset(zcol[:], 0.0)
        for pb in p2:
            dst = bass.AP(tensor=pb, offset=0,
                          ap=[[fs + 1, 128], [(fs + 1) * 128, zc]])
            nc.sync.dma_start(out=dst, in_=zcol[:])

        # ---- persistent activations ----
        qTu_t = persist.tile([128, not_ * cs], BF16)
        qTv_t = persist.tile([128, not_ * cs], BF16)
        kT_t = persist.tile([128, not_ * fs], BF16)
        rT_t = persist.tile([128, not_ * fs], BF16)
        qTu = qTu_t[:].rearrange("p (ot i) -> p ot i", ot=not_)
        qTv = qTv_t[:].rearrange("p (ot i) -> p ot i", ot=not_)
        kT = kT_t[:].rearrange("p (ot j) -> p ot j", ot=not_)
        rT = rT_t[:].rearrange("p (ot j) -> p ot j", ot=not_)
        vT_t = persist.tile([128, not_ * fs], BF16)
        vT = vT_t[:].rearrange("p (ot j) -> p ot j", ot=not_)
        ofin_t = persist.tile([128, not_ * cs], BF16)
        ofin = ofin_t[:].rearrange("p (kt i) -> p kt i", kt=not_)

        Ident = mybir.ActivationFunctionType.Identity
        Exp = mybir.ActivationFunctionType.Exp
        DR = mybir.MatmulPerfMode.DoubleRow
        _abl = os.environ.get("KABL", "full")
        bdum = None
        if _abl in ("noshift", "noread"):
            bdum = persist.tile([128, cs], BF16, name="bdum", tag="bdum")
            nc.vector.memset(bdum[:], 0.0)
        if _abl == "noattn":
            nc.vector.memset(ofin_t[:], 0.0)

        def _phases():
            io_eng = (nc.sync if os.environ.get("KQIO", "1") == "0"
                      else nc.gpsimd)
            # ---------- projections ----------
            # q: evict twice with +u / +v bias.
            for s0 in range(0, cs, 512):
                xq = []
                for kk in range(nk):
                    t = xqp.tile([128, 512], BF16, name="xq", tag="xq")
                    io_eng.dma_start(
                        out=t[:],
                        in_=xcur[kk * 128:(kk + 1) * 128, s0:s0 + 512])
                    xq.append(t)
                for ot in range(not_):
                    ps = psA.tile([128, 512], F32, name="psq", tag="a")
                    for kk in range(nk):
                        nc.tensor.matmul(ps[:],
                                         wq[:, kk, ot * 128:(ot + 1) * 128],
                                         xq[kk][:, :],
                                         start=(kk == 0), stop=(kk == nk - 1))
                    nc.scalar.activation(qTu[:, ot, s0:s0 + 512], ps[:], Ident,
                                         bias=u_sb[:, ot:ot + 1])
                    nc.scalar.activation(qTv[:, ot, s0:s0 + 512], ps[:], Ident,
                                         bias=v_sb[:, ot:ot + 1])

            # r: token-chunked loads
            for c0 in range(0, fs, 1024):
                xr = []
                for kk in range(nk):
                    t = xrp.tile([128, 1024], BF16, name="xr", tag="xr")
                    io_eng.dma_start(
                        out=t[:],
                        in_=posT[kk * 128:(kk + 1) * 128, c0:c0 + 1024])
                    xr.append(t)
                for s0 in range(0, 1024, 512):
                    for ot in range(not_):
                        ps = psA.tile([128, 512], F32, name="psr", tag="a")
                        for kk in range(nk):
                            nc.tensor.matmul(
                                ps[:], wr[:, kk, ot * 128:(ot + 1) * 128],
                                xr[kk][:, s0:s0 + 512],
                                start=(kk == 0), stop=(kk == nk - 1))
                        nc.vector.tensor_copy(rT[:, ot, c0 + s0:c0 + s0 + 512],
                                              ps[:])

            # k and v share the x_full chunk loads
            for c0 in range(0, fs, 1024):
                xf = []
                for kk in range(nk):
                    t = xkv.tile([128, 1024], BF16, name="xf", tag="xf")
                    io_eng.dma_start(
                        out=t[:],
                        in_=xfull[kk * 128:(kk + 1) * 128, c0:c0 + 1024])
                    xf.append(t)
                for s0 in range(0, 1024, 512):
                    for ot in range(not_):
                        psk = psA.tile([128, 512], F32, name="psk", tag="a")
                        psv = psB.tile([128, 512], F32, name="psv", tag="b")
                        for kk in range(nk):
                            nc.tensor.matmul(
                                psk[:], wk[:, kk, ot * 128:(ot + 1) * 128],
                                xf[kk][:, s0:s0 + 512],
                                start=(kk == 0), stop=(kk == nk - 1))
                            nc.tensor.matmul(
                                psv[:], wv[:, kk, ot * 128:(ot + 1) * 128],
                                xf[kk][:, s0:s0 + 512],
                                start=(kk == 0), stop=(kk == nk - 1))
                        nc.scalar.copy(kT[:, ot, c0 + s0:c0 + s0 + 512],
                                       psk[:])
                        nc.vector.tensor_copy(
                            vT[:, ot, c0 + s0:c0 + s0 + 512], psv[:])

            # ---------- attention, software-pipelined one pair deep ------
            # Cross-queue DRAM ordering is synchronized by the tile
            # framework (verified: the transposed reads carry sem waits on
            # the swdge write-completion semaphores), so p2 writes can go
            # on the idle gpsimd queue. KWQ=0 forces them back onto SP.
            wr_eng = nc.sync if os.environ.get("KWQ", "1") == "0" \
                else nc.gpsimd
            _krq = int(os.environ.get("KRQ", "1"))

            def bd_step(h, it):
                # one i-tile of pair h's BD: 4 matmuls, evictions, p2 write
                t_, r0 = h // 2, (h % 2) * hd
                hs = slice(r0, r0 + hd)
                pb = p2[h % 4]
                st = sts.tile([128, fs], BF16)
                for jc in range(NJC):
                    psbd = psB.tile([128, 512], F32, name="psbd", tag="b")
                    nc.tensor.matmul(
                        psbd[:],
                        qTv[hs, t_, it * 128:(it + 1) * 128],
                        rT[hs, t_, jc * 512:(jc + 1) * 512],
                        start=True, stop=True)
                    sl = st[:, jc * 512:(jc + 1) * 512]
                    if jc % 2 == 0:
                        nc.vector.tensor_copy(sl, psbd[:])
                    else:
                        nc.scalar.copy(sl, psbd[:])
                if _abl not in ("noshift", "nowrite"):
                    dst = bass.AP(
                        tensor=pb,
                        offset=(it * 128) * (fs + 1) + 1,
                        ap=[[fs + 1, 128], [1, fs]])
                    wr_eng.dma_start(out=dst, in_=st[:])

            def bd_reads(h):
                if _abl in ("noshift", "noread"):
                    return [bdum] * NJ
                pb = p2[h % 4]
                # NOTE: consolidating 4 j-tiles into one 3D-out DmaTranspose
                # matches CoreSim semantics but produces NaN on HW (walrus
                # lowering maps the extra out dim differently) — keep
                # per-tile reads.
                bds = []
                for jt in range(NJ):
                    bt = bdst.tile([128, cs], BF16)
                    srcap = bass.AP(tensor=pb, offset=cs + jt * 128,
                                    ap=[[fs, cs], [1, 128]])
                    rd_eng = nc.scalar if (_krq == 2 and jt % 2) else nc.sync
                    rd_eng.dma_start(out=bt[:], in_=srcap, transpose=True)
                    bds.append(bt)
                return bds

            def emit_attn(h, bds, nxt):
                t_, r0 = h // 2, (h % 2) * hd
                hs = slice(r0, r0 + hd)
                po = [psO.tile([65, 512], F32, name=f"pso{ic}", tag=f"o{ic}")
                      for ic in range(NIC)]
                ilv = os.environ.get("KILV", "1") == "1"
                if nxt is not None and not ilv:
                    for it in range(NI):
                        bd_step(nxt, it)
                for jt in range(NJ):
                    # interleave next pair's BD work to keep PE queue full
                    if nxt is not None and ilv and jt % 2 == 0:
                        bd_step(nxt, jt // 2)
                    pvx = psX.tile([128, 64], BF16, name="pvx", tag="x")
                    nc.tensor.transpose(
                        pvx[:], vT[hs, t_, jt * 128:(jt + 1) * 128],
                        ident[hs, r0:r0 + hd])
                    vx = vxp.tile([128, 65], BF16)
                    nc.vector.tensor_copy(vx[:, 0:hd], pvx[:])
                    nc.vector.memset(vx[:, hd:hd + 1], 1.0)
                    et = ea.tile([128, cs], BF16)
                    for ic in range(NIC):
                        psac = psA.tile([128, 512], F32, name="psac", tag="a")
                        nc.tensor.matmul(
                            psac[:],
                            kT[hs, t_, jt * 128:(jt + 1) * 128],
                            qTu[hs, t_, ic * 512:(ic + 1) * 512],
                            start=True, stop=False)
                        nc.tensor.matmul(
                            psac[:], ident[:, :],
                            bds[jt][:, ic * 512:(ic + 1) * 512],
                            start=False, stop=True)
                        nc.scalar.activation(
                            et[:, ic * 512:(ic + 1) * 512], psac[:],
                            Exp, scale=scale)
                    for ic in range(NIC):
                        nc.tensor.matmul(
                            po[ic][:], vx[:, 0:65],
                            et[:, ic * 512:(ic + 1) * 512],
                            start=(jt == 0), stop=(jt == NJ - 1))
                for ic in range(NIC):
                    ov = onrm.tile([65, 512], F32)
                    nc.vector.tensor_copy(ov[:], po[ic][:])
                    rc = onrm.tile([1, 512], F32)
                    nc.vector.reciprocal(rc[:], ov[hd:hd + 1, :])
                    rb = onrm.tile([hd, 512], F32)
                    nc.gpsimd.partition_broadcast(rb[:], rc[:])
                    nc.vector.tensor_mul(
                        ofin[r0:r0 + hd, t_, ic * 512:(ic + 1) * 512],
                        ov[0:hd, :], rb[:])

            for it in range(NI):
                bd_step(0, it)
            prev = bd_reads(0)
            for h in range(hpc):
                nxt = h + 1 if h + 1 < hpc else None
                if _abl != "noattn":
                    emit_attn(h, prev, nxt)
                elif nxt is not None:
                    for it in range(NI):
                        bd_step(nxt, it)
                prev = bd_reads(nxt) if nxt is not None else None

            # ---------- output projection ----------
            for oc in range(d // 128):
                yt = yout.tile([128, cs], BF16)
                for ic in range(NIC):
                    psy = psX.tile([128, 512], F32, name="psy", tag="x")
                    for kt in range(not_):
                        nc.tensor.matmul(
                            psy[:], wo[:, kt, oc * 128:(oc + 1) * 128],
                            ofin[:, kt, ic * 512:(ic + 1) * 512],
                            start=(kt == 0), stop=(kt == not_ - 1))
                    nc.vector.tensor_copy(yt[:, ic * 512:(ic + 1) * 512],
                                          psy[:])
                io_eng.dma_start(
                    out=y_d[oc * 128:(oc + 1) * 128, :], in_=yt[:])

        for _rep in range(loop):
            _phases()

    nc.compile()
    return nc


_NC_CACHE = {}


def _get_nc(dims):
    if dims not in _NC_CACHE:
        _NC_CACHE[dims] = build_core_kernel(*dims)
    return _NC_CACHE[dims]


def make_in_maps(inputs, pos_embedding, full_input, u, v, Wkv, Wq, Wr, Wo,
                 cs=CS, fs=FS, bs=BS, d=D_MODEL, hpc=HPC, hd=HD,
                 n_cores=N_CORES):
    dc = hpc * hd
    inputs = np.asarray(inputs, np.float32)        # (cs, bs, d)
    full_input = np.asarray(full_input, np.float32)
    posT = np.ascontiguousarray(
        np.asarray(pos_embedding, np.float32).T).astype(NPBF16)
    Wkv = np.asarray(Wkv, np.float32)
    Wq = np.asarray(Wq, np.float32)
    Wr = np.asarray(Wr, np.float32)
    Wo = np.asarray(Wo, np.float32)
    u = np.asarray(u, np.float32).reshape(-1)      # (H*HD,)
    v = np.asarray(v, np.float32).reshape(-1)

    xcurT = [np.ascontiguousarray(inputs[:, b, :].T).astype(NPBF16)
             for b in range(bs)]
    xfullT = [np.ascontiguousarray(full_input[:, b, :].T).astype(NPBF16)
              for b in range(bs)]

    in_maps = []
    for c in range(n_cores):
        b, g = c // 2, c % 2
        cols = slice(g * dc, (g + 1) * dc)
        in_maps.append({
            "xcur": xcurT[b],
            "xfull": xfullT[b],
            "posT": posT,
            "wq": np.ascontiguousarray(Wq[:, cols]).astype(NPBF16),
            "wk": np.ascontiguousarray(
                Wkv[:, g * dc:(g + 1) * dc]).astype(NPBF16),
            "wr": np.ascontiguousarray(Wr[:, cols]).astype(NPBF16),
            "wv": np.ascontiguousarray(
                Wkv[:, d + g * dc:d + (g + 1) * dc]).astype(NPBF16),
            "wo": np.ascontiguousarray(Wo[g * dc:(g + 1) * dc, :]
                                       ).astype(NPBF16),
            "u": np.ascontiguousarray(
                u[g * dc:(g + 1) * dc].reshape(dc, 1)).astype(np.float32),
            "v": np.ascontiguousarray(
                v[g * dc:(g + 1) * dc].reshape(dc, 1)).astype(np.float32),
        })
    return in_maps


def combine_outputs(results, bo, cs=CS, bs=BS, d=D_MODEL):
    out = np.empty((cs, bs, d), np.float32)
    for b in range(bs):
        acc = (np.asarray(results[2 * b]["y"], np.float32)
               + np.asarray(results[2 * b + 1]["y"], np.float32))
        out[:, b, :] = acc.T
    return (out + np.asarray(bo, np.float32)[None, None, :]).astype(np.float32)


def _build_runner(nc, n_cores, reps=1):
    """jit-compiled sharded executor for the prebuilt bass module (cached)."""
    import jax
    from jax.sharding import Mesh, PartitionSpec, NamedSharding
    from jax.experimental.shard_map import shard_map
    from concourse import bass2jax

    bass2jax.install_neuronx_cc_hook()
    partition_name = (nc.partition_id_tensor.name
                      if nc.partition_id_tensor else None)
    in_names, out_names, out_avals, zero_outs = [], [], [], []
    for alloc in nc.m.functions[0].allocations:
        if not isinstance(alloc, mybir.MemoryLocationSet):
            continue
        name = alloc.memorylocations[0].name
        if alloc.kind == "ExternalInput":
            if name != partition_name:
                in_names.append(name)
        elif alloc.kind == "ExternalOutput":
            shape = tuple(alloc.tensor_shape)
            dtype = mybir.dt.np(alloc.dtype)
            out_names.append(name)
            out_avals.append(jax.core.ShapedArray(shape, dtype))
            zero_outs.append(np.zeros(shape, dtype))
    n_params = len(in_names)
    all_names = list(in_names) + list(out_names)
    if partition_name is not None:
        all_names.append(partition_name)

    def _body(*args):
        outs = None
        for _ in range(reps):
            operands = list(args)
            if partition_name is not None:
                operands.append(bass2jax.partition_id_tensor())
            outs = bass2jax._bass_exec_p.bind(
                *operands,
                out_avals=tuple(out_avals),
                in_names=tuple(all_names),
                out_names=tuple(out_names),
                lowering_input_output_aliases=(),
                sim_require_finite=True,
                sim_require_nnan=True,
                nc=nc,
            )
        return tuple(outs)

    devices = jax.devices()[:n_cores]
    mesh = Mesh(np.asarray(devices), ("core",))
    n_outs = len(out_avals)
    fn = jax.jit(
        shard_map(_body, mesh=mesh,
                  in_specs=(PartitionSpec("core"),) * (n_params + n_outs),
                  out_specs=(PartitionSpec("core"),) * n_outs,
                  check_rep=False),
        keep_unused=True)
    sharding = NamedSharding(mesh, PartitionSpec("core"))

    def runner(in_maps):
        import jax as _jax
        per_core = [[np.asarray(m[name]) for name in in_names] for m in in_maps]
        args = [np.concatenate([per_core[c][i] for c in range(n_cores)], axis=0)
                for i in range(n_params)]
        args += [np.zeros((n_cores * z.shape[0], *z.shape[1:]), z.dtype)
                 for z in zero_outs]
        placed = [_jax.device_put(a, sharding) for a in args]
        out = fn(*placed)
        _jax.block_until_ready(out)
        return [
            {name: np.asarray(out[i]).reshape(n_cores, *out_avals[i].shape)[c]
             for i, name in enumerate(out_names)}
            for c in range(n_cores)
        ]

    return runner


_RUNNER_CACHE = {}


def _get_runner(dims):
    if dims not in _RUNNER_CACHE:
        nc = _get_nc(dims)
        _RUNNER_CACHE[dims] = _build_runner(nc, N_CORES)
    return _RUNNER_CACHE[dims]


def kernel(**inputs):
    dims = (CS, FS, BS, D_MODEL, HPC, HD)
    runner = _get_runner(dims)
    in_maps = make_in_maps(
        inputs["inputs"], inputs["pos_embedding"], inputs["full_input"],
        inputs["u"], inputs["v"], inputs["Wkv"], inputs["Wq"], inputs["Wr"],
        inputs["Wo"])
    results = runner(in_maps)
    return combine_outputs(results, inputs["bo"])
